# revision 50
# baseline (speedup 1.0000x reference)
"""Trainium2 Bass kernel for the nn_Ensemble_net MoE-routing problem.

Strategy: data-parallel over batch B=4096 across 8 NeuronCores (512 rows each).
Per core, y_pred is streamed once in 64 tiles laid out [(8 samples x 16
experts) partitions, 1000 classes]:
  - ACT computes exp with fused free-dim accumulation (-> logsumexp).
  - GPSIMD indirect_copy gathers y[b, m, labels[b]] 16 tiles at a time (each
    16-partition group shares one sample per index slot).
  - PE computes both weighted combines (ems_out / ems_out_post); 8 tiles
    accumulate into one [128, C] PSUM tile (64 b-major ens rows + 64 b-major
    post rows) via zero-padded block-diagonal stationary operands, so the
    PSUM->SBUF eviction costs one DVE copy per 8 tiles.
Routing MLP (w1/w2/w3) runs on PE in transposed space; rowwise softmax /
top-k / loss math runs on [128,16] b-major tiles (DVE max8 for top-k).
Losses are reduced to per-core partial sums; the host combines them.

build_nc(niter=N) emits the whole body N times into one NEFF — used by the
differential timing harness (dispatch overhead through the axon tunnel is
~90 ms, far larger than the kernel itself).
"""

import os
import numpy as np
from contextlib import ExitStack

ABLATE = set(os.environ.get("KERNEL_ABLATE", "").split(","))

import concourse.bass as bass
import concourse.bacc as bacc
import concourse.tile as tile
from concourse import mybir
from concourse.bass_utils import run_bass_kernel_spmd

F32 = mybir.dt.float32
U16 = mybir.dt.uint16
AX = mybir.AxisListType
OP = mybir.AluOpType
AF = mybir.ActivationFunctionType

B, M, C, NUM_LOCAL, EMB, D = 4096, 16, 1000, 4, 100, 1000
NCORES = 8
R = B // NCORES          # 512 rows per core
T = R // 8               # 64 main-loop tiles per core
G = R // 128             # 4 b-major groups per core
DCH = 8                  # contraction chunks for D=1000 (125 partitions each)
DP = D // DCH            # 125
EPS = 1e-12


def build_nc(niter=1):
    nc = bacc.Bacc("TRN2", target_bir_lowering=False, debug=False)

    # ---- DRAM I/O ----
    xT = nc.dram_tensor("xT", [D, R], F32, kind="ExternalInput")
    yy = nc.dram_tensor("yy", [R, M * C], F32, kind="ExternalInput")
    w1T = nc.dram_tensor("w1T", [D, EMB], F32, kind="ExternalInput")
    w2T = nc.dram_tensor("w2T", [D, EMB], F32, kind="ExternalInput")
    membT = nc.dram_tensor("membT", [D, M], F32, kind="ExternalInput")
    w3T = nc.dram_tensor("w3T", [M, M], F32, kind="ExternalInput")
    b1 = nc.dram_tensor("b1", [EMB, 1], F32, kind="ExternalInput")
    b2 = nc.dram_tensor("b2", [EMB, 1], F32, kind="ExternalInput")
    b3 = nc.dram_tensor("b3", [M, 1], F32, kind="ExternalInput")
    iota_c = nc.dram_tensor("iota_c", [128, C], F32, kind="ExternalInput")
    lab_u16 = nc.dram_tensor("lab_u16", [128, T // 8], U16, kind="ExternalInput")
    lab_g64 = nc.dram_tensor("lab_g64", [64, T // 8], F32, kind="ExternalInput")
    ident = nc.dram_tensor("ident", [128, 128], F32, kind="ExternalInput")
    bdm = nc.dram_tensor("bdm", [128, 8, 64], F32, kind="ExternalInput")

    post_out = nc.dram_tensor("post_out", [R, C], F32, kind="ExternalOutput")
    wms_out = nc.dram_tensor("wms_out", [R, M], F32, kind="ExternalOutput")
    tc_out = nc.dram_tensor("tc_out", [R, M], F32, kind="ExternalOutput")
    partials = nc.dram_tensor("partials", [1, 8], F32, kind="ExternalOutput")

    with tile.TileContext(nc) as tc, ExitStack() as ctx:
        singles = ctx.enter_context(tc.tile_pool(name="singles", bufs=1))
        small = ctx.enter_context(tc.tile_pool(name="small", bufs=4))
        dram = ctx.enter_context(tc.tile_pool(name="dram", bufs=1, space="DRAM"))

        # ---- constants (loaded once) ----
        iota_sb = singles.tile([128, C], F32)
        nc.sync.dma_start(out=iota_sb, in_=iota_c[:])
        lab16_sb = singles.tile([128, T // 8], U16)
        nc.sync.dma_start(out=lab16_sb, in_=lab_u16[:])
        labg64_sb = singles.tile([64, T // 8], F32)
        nc.sync.dma_start(out=labg64_sb, in_=lab_g64[:])
        ident_sb = singles.tile([128, 128], F32)
        nc.sync.dma_start(out=ident_sb, in_=ident[:])
        bdm_sb = singles.tile([128, 8, 64], F32)
        nc.sync.dma_start(out=bdm_sb, in_=bdm[:])
        b1_sb = singles.tile([EMB, 1], F32)
        nc.sync.dma_start(out=b1_sb, in_=b1[:])
        b2_sb = singles.tile([EMB, 1], F32)
        nc.sync.dma_start(out=b2_sb, in_=b2[:])
        b3_sb = singles.tile([M, 1], F32)
        nc.sync.dma_start(out=b3_sb, in_=b3[:])
        w3_sb = singles.tile([M, M], F32)
        nc.sync.dma_start(out=w3_sb, in_=w3T[:])

        for it in range(niter):
            emit_body(nc, tc, it, singles, small, dram,
                      iota_sb, lab16_sb, labg64_sb, ident_sb, bdm_sb,
                      b1_sb, b2_sb, b3_sb, w3_sb,
                      xT, yy, membT, w1T, w2T,
                      post_out, wms_out, tc_out, partials)

    nc.compile()
    return nc


def emit_body(nc, tc, it, singles, small, dram,
              iota_sb, lab16_sb, labg64_sb, ident_sb, bdm_sb,
              b1_sb, b2_sb, b3_sb, w3_sb,
              xT, yy, membT, w1T, w2T,
              post_out, wms_out, tc_out, partials):
    # ---- per-iteration persistent accumulators ----
    sume = singles.tile([128, T], F32)    # sum(exp(y)) per (sample, expert)
    ytr = singles.tile([128, T], F32)     # y[b, m, labels[b]]
    etrue = singles.tile([64, T // 8], F32)   # ems_out[b, labels[b]]
    esum = singles.tile([64, T // 8], F32)    # sum(exp(ems_out[b, :]))
    wms_all = singles.tile([128, G, M], F32)
    tc_all = singles.tile([128, G, M], F32)
    wmcol = singles.tile([128, T], F32)
    pwcol = singles.tile([128, T], F32)

    # =========================================================
    # Phase 1: routing MLP in transposed space
    #   featT = relu(w1 @ x.T + b1)    [EMB, R]
    #   embT  = relu(w2 @ memb.T + b2) [EMB, M]
    #   scoresT[m, b], weightsT = softplus(w3 @ scoresT + b3) [M, R]
    # The streaming SBUF pools open FIRST so y prefetch / exp / gather can
    # run concurrently with routing (only the combine matmuls wait on
    # wmcol/pwcol). PSUM pools stay phase-scoped (8-bank budget).
    # =========================================================
    _mainpools = [
        tc.tile_pool(name=f"ypool{it}", bufs=2),
        tc.tile_pool(name=f"epool{it}", bufs=2),
        tc.tile_pool(name=f"bdpool{it}", bufs=3),
        tc.tile_pool(name=f"cbpool{it}", bufs=2),
    ]
    ypool, epool, bdpool, cbpool = [p.__enter__() for p in _mainpools]
    with tc.tile_pool(name=f"routing{it}", bufs=1) as routing, \
         tc.tile_pool(name=f"rpsum{it}", bufs=1, space="PSUM") as rpsum, \
         tc.tile_pool(name=f"tpsum{it}", bufs=2, space="PSUM") as spsum:
        x_sb = routing.tile([DP, DCH, R], F32)
        nc.sync.dma_start(
            out=x_sb, in_=xT[:].rearrange("(c p) r -> p c r", p=DP))
        w1_sb = routing.tile([DP, DCH, EMB], F32)
        nc.sync.dma_start(
            out=w1_sb, in_=w1T[:].rearrange("(c p) e -> p c e", p=DP))
        w2_sb = routing.tile([DP, DCH, EMB], F32)
        nc.sync.dma_start(
            out=w2_sb, in_=w2T[:].rearrange("(c p) e -> p c e", p=DP))
        me_sb = routing.tile([DP, DCH, M], F32)
        nc.sync.dma_start(
            out=me_sb, in_=membT[:].rearrange("(c p) m -> p c m", p=DP))

        ps_feat = rpsum.tile([EMB, R], F32)
        for c in range(DCH):
            nc.tensor.matmul(ps_feat, w1_sb[:, c, :], x_sb[:, c, :],
                             start=(c == 0), stop=(c == DCH - 1))
        featT = routing.tile([EMB, R], F32)
        nc.vector.tensor_scalar(featT, ps_feat, b1_sb[:], 0.0, OP.add, OP.max)

        ps_emb = rpsum.tile([EMB, M], F32)
        for c in range(DCH):
            nc.tensor.matmul(ps_emb, w2_sb[:, c, :], me_sb[:, c, :],
                             start=(c == 0), stop=(c == DCH - 1))
        embT = routing.tile([EMB, M], F32)
        nc.vector.tensor_scalar(embT, ps_emb, b2_sb[:], 0.0, OP.add, OP.max)

        ps_sc = rpsum.tile([M, R], F32)
        nc.tensor.matmul(ps_sc, embT, featT, start=True, stop=True)
        scoresT = routing.tile([M, R], F32)
        nc.vector.tensor_copy(out=scoresT, in_=ps_sc)

        ps_u = rpsum.tile([M, R], F32)
        nc.tensor.matmul(ps_u, w3_sb, scoresT, start=True, stop=True)
        u_sb = routing.tile([M, R], F32)
        nc.vector.tensor_scalar(u_sb, ps_u, b3_sb[:], None, OP.add)

        # stable softplus(u) = relu(u) + ln(1 + exp(-|u|))
        nu = routing.tile([M, R], F32)
        nc.vector.tensor_scalar(nu, u_sb, -1.0, None, OP.mult)
        au = routing.tile([M, R], F32)
        nc.vector.tensor_tensor(out=au, in0=u_sb, in1=nu, op=OP.max)
        eau = routing.tile([M, R], F32)
        nc.scalar.activation(eau, au, AF.Exp, scale=-1.0)
        l1p = routing.tile([M, R], F32)
        nc.scalar.activation(l1p, eau, AF.Ln, bias=1.0)
        ru = routing.tile([M, R], F32)
        nc.vector.tensor_scalar(ru, u_sb, 0.0, None, OP.max)
        weightsT = routing.tile([M, R], F32)
        nc.vector.tensor_tensor(out=weightsT, in0=ru, in1=l1p, op=OP.add)

        # =====================================================
        # Phase 2: rowwise weight math on b-major [128, M] tiles
        # =====================================================
        scr_w = dram.tile([R, M], F32)
        scr_p = dram.tile([R, M], F32)
        for bt in range(G):
            ps_t = spsum.tile([128, M], F32, tag="tps")
            nc.tensor.transpose(
                ps_t, weightsT[:, bt * 128:(bt + 1) * 128], ident_sb[:M, :M])
            w_b = small.tile([128, M], F32)
            nc.vector.tensor_copy(out=w_b, in_=ps_t)

            srow = small.tile([128, 1], F32, tag="srow")
            nc.vector.tensor_reduce(srow, w_b, AX.X, OP.add)
            nc.vector.tensor_scalar(srow, srow, EPS, None, OP.max)
            rs = small.tile([128, 1], F32, tag="rs")
            nc.vector.reciprocal(rs, srow)
            wm_b = small.tile([128, M], F32, tag="wm_b")
            nc.vector.tensor_scalar(wm_b, w_b, rs[:], None, OP.mult)
            nc.sync.dma_start(out=scr_w[bt * 128:(bt + 1) * 128, :], in_=wm_b)

            # top-4 sparsification via max8 threshold
            top8 = small.tile([128, 8], F32, tag="top8")
            nc.vector.max(top8, w_b)
            pwr = small.tile([128, M], F32, tag="pwr")
            spost = small.tile([128, 1], F32, tag="spost")
            nc.vector.scalar_tensor_tensor(
                out=pwr, in0=w_b, scalar=top8[:, NUM_LOCAL - 1:NUM_LOCAL],
                in1=w_b, op0=OP.is_ge, op1=OP.mult, accum_out=spost)
            nc.vector.tensor_scalar(spost, spost, EPS, None, OP.max)
            rsp = small.tile([128, 1], F32, tag="rsp")
            nc.vector.reciprocal(rsp, spost)
            pw_b = small.tile([128, M], F32, tag="pw_b")
            nc.vector.tensor_scalar(pw_b, pwr, rsp[:], None, OP.mult)
            nc.sync.dma_start(out=scr_p[bt * 128:(bt + 1) * 128, :], in_=pw_b)

            # wm_s = softmax(weighted_mat) over experts
            ewm = small.tile([128, M], F32, tag="ewm")
            sew = small.tile([128, 1], F32, tag="sew")
            nc.scalar.activation(ewm, wm_b, AF.Exp, accum_out=sew)
            rsew = small.tile([128, 1], F32, tag="rsew")
            nc.vector.reciprocal(rsew, sew)
            nc.vector.tensor_scalar(
                wms_all[:, bt, :], ewm, rsew[:], None, OP.mult)

        nc.sync.dma_start(
            out=wms_out[:].rearrange("(g p) m -> p g m", p=128),
            in_=wms_all)

        # relayout weighted_mat / post_weights into per-tile PE columns:
        # wmcol[(b8*16+m), t] = wm[t*8+b8, m]
        for src, dst in ((scr_w, wmcol), (scr_p, pwcol)):
            v = routing.tile([T, 128], F32, tag="wcolv")
            nc.sync.dma_start(
                out=v, in_=src[:].rearrange("(t x) m -> t (x m)", t=T))
            ps_c = spsum.tile([128, T], F32, tag="tps")
            nc.tensor.transpose(ps_c, v, ident_sb[:T, :T])
            nc.vector.tensor_copy(out=dst, in_=ps_c)

    # =========================================================
    # Phase 3: main streaming loop over y, 4 super-tiles of 16 tiles;
    # each half-super-tile (8 tiles) accumulates its combines into one
    # [128, C] PSUM tile: rows 0:64 ens (b-major), 64:128 post (b-major).
    # =========================================================
    with tc.tile_pool(name=f"ypool{it}", bufs=2) as ypool, \
         tc.tile_pool(name=f"epool{it}", bufs=3) as epool, \
         tc.tile_pool(name=f"bdpool{it}", bufs=3) as bdpool, \
         tc.tile_pool(name=f"cpsum{it}", bufs=4, space="PSUM") as cpsum, \
         tc.tile_pool(name=f"cbpool{it}", bufs=2) as cbpool:
        for sb in range(T // 16):
            ysup = ypool.tile([128, 16, C], F32)
            # 4 DMA slices for pipelining (2MB each)
            for q in range(4):
                nc.sync.dma_start(
                    out=ysup[:, q * 4:(q + 1) * 4, :],
                    in_=yy[sb * 128 + q * 32:sb * 128 + (q + 1) * 32, :]
                    .rearrange("(j b) (m c) -> (b m) j c", j=4, c=C))

            # label gather, 16 tiles at once: one index per partition of
            # each 16-partition group; index j gathers tile j's value.
            if "gather" not in ABLATE:
                nc.gpsimd.indirect_copy(
                    out=ytr[:, sb * 16:(sb + 1) * 16],
                    data=ysup.rearrange("p j c -> p (j c)"),
                    idxs=lab16_sb[:, 2 * sb:2 * sb + 1],
                    i_know_ap_gather_is_preferred=True)
            else:
                nc.vector.memset(ytr[:, sb * 16:(sb + 1) * 16], 0.5)

            for h2 in range(2):   # psum batches of 8 tiles
                h = sb * 2 + h2
                cps = cpsum.tile([128, C], F32)
                for jj in range(8):
                    j = h2 * 8 + jj
                    t = sb * 16 + j
                    y_t = ysup[:, j, :]

                    if "comb" not in ABLATE:
                        # per-tile stationary operand, zero outside its
                        # columns: ens jj*8:(jj+1)*8, post 64+jj*8:...
                        bd = bdpool.tile([128, 128], F32)
                        nc.vector.tensor_scalar(
                            bd[:, 0:64], bdm_sb[:, jj, :],
                            wmcol[:, t:t + 1], None, OP.mult)
                        nc.vector.tensor_scalar(
                            bd[:, 64:128], bdm_sb[:, jj, :],
                            pwcol[:, t:t + 1], None, OP.mult)

                        nc.tensor.matmul(cps[:, 0:512], bd, y_t[:, 0:512],
                                         start=(jj == 0), stop=(jj == 7),
                                         skip_group_check=True)
                        nc.tensor.matmul(cps[:, 512:C], bd, y_t[:, 512:C],
                                         start=(jj == 0), stop=(jj == 7),
                                         skip_group_check=True)
                    elif jj == 0:
                        nc.vector.memset(cps, 0.25)

                    # exp + fused row-sum (for logsumexp)
                    if "exp" not in ABLATE:
                        esc = epool.tile([128, C], F32, tag="esc")
                        nc.scalar.activation(esc, y_t, AF.Exp,
                                             accum_out=sume[:, t:t + 1])
                    elif jj == 0:
                        nc.vector.memset(sume[:, t:t + 8], 1.0)

                # evict the batch: one copy for 64 ens + 64 post rows
                comb = cbpool.tile([128, C], F32)
                if "evict" in ABLATE:
                    nc.vector.memset(comb[:, 0:8], 0.25)
                else:
                    nc.vector.tensor_copy(out=comb, in_=cps)
                    nc.sync.dma_start(
                        out=post_out[h * 64:(h + 1) * 64, :],
                        in_=comb[64:128, :])

                # ensemble-loss pieces on the 64 b-major ens rows
                esc2 = epool.tile([128, C], F32, tag="esc")
                nc.vector.scalar_tensor_tensor(
                    out=esc2[0:64, :], in0=iota_sb[0:64, :],
                    scalar=labg64_sb[:, h:h + 1],
                    in1=comb[0:64, :], op0=OP.is_equal, op1=OP.mult,
                    accum_out=etrue[:, h:h + 1])
                esc3 = epool.tile([128, C], F32, tag="esc")
                nc.scalar.activation(esc3[0:64, :], comb[0:64, :], AF.Exp,
                                     accum_out=esum[:, h:h + 1])

    for p_ in reversed(_mainpools):
        p_.__exit__(None, None, None)

    # =========================================================
    # Phase 4: tail — logsumexp, relayout, losses
    # =========================================================
    if "tail" in ABLATE:   # timing-ablation stub, never used in production
        junkt = small.tile([128, 8], F32, tag="junkt")
        nc.vector.tensor_copy(out=junkt[:, 0:1], in_=sume[:, 0:1])
        nc.vector.tensor_copy(out=junkt[:, 1:2], in_=ytr[:, 0:1])
        nc.sync.dma_start(out=partials[:], in_=junkt[0:1, :])
        return
    _zpsum_cm = tc.tile_pool(name=f"zpsum{it}", bufs=1, space="PSUM")
    spsum = _zpsum_cm.__enter__()
    pk = singles.tile([128, 128], F32)
    nc.scalar.activation(pk[:, 0:T], sume, AF.Ln)
    nc.vector.tensor_copy(out=pk[:, T:2 * T], in_=ytr)
    ps_pk = spsum.tile([128, 128], F32, tag="tps")
    nc.tensor.transpose(ps_pk, pk, ident_sb)
    pkt = singles.tile([128, 128], F32)
    nc.vector.tensor_copy(out=pkt, in_=ps_pk)
    scr_pk = dram.tile([128, 128], F32)
    nc.sync.dma_start(out=scr_pk, in_=pkt)

    lse_bm = singles.tile([128, G, M], F32)
    nc.sync.dma_start(
        out=lse_bm,
        in_=scr_pk[0:T, :].rearrange(
            "(g th) (bl m) -> (th bl) g m", g=G, bl=8))
    ytr_bm = singles.tile([128, G, M], F32)
    nc.sync.dma_start(
        out=ytr_bm,
        in_=scr_pk[T:2 * T, :].rearrange(
            "(g th) (bl m) -> (th bl) g m", g=G, bl=8))

    # batched [128, G*M] tail math; loss partials accumulate straight into
    # per-partition columns of part8 via fused accum_out over all 64 cols
    part8 = singles.tile([128, 8], F32)
    nc.vector.memset(part8, 0.0)
    lse_f = lse_bm.rearrange("p g m -> p (g m)")
    ytr_f = ytr_bm.rearrange("p g m -> p (g m)")
    wms_f = wms_all.rearrange("p g m -> p (g m)")

    epl = small.tile([128, G * M], F32, tag="epl")
    nc.vector.tensor_tensor(out=epl, in0=lse_f, in1=ytr_f, op=OP.subtract)
    junk = small.tile([128, G * M], F32, tag="junk")
    nc.vector.scalar_tensor_tensor(
        out=junk, in0=epl, scalar=1.0, in1=wms_f,
        op0=OP.mult, op1=OP.mult, accum_out=part8[:, 0:1])

    dd = small.tile([128, G * M], F32, tag="dd")
    nc.vector.tensor_tensor(out=dd, in0=ytr_f, in1=lse_f, op=OP.subtract)
    tcp = small.tile([128, G * M], F32, tag="tcp")
    nc.scalar.activation(tcp, dd, AF.Exp)

    # true_confs = softmax(tcp) over each 16-col group
    etc = small.tile([128, G, M], F32, tag="etc")
    nc.scalar.activation(etc.rearrange("p g m -> p (g m)"), tcp, AF.Exp)
    s1 = small.tile([128, G], F32, tag="s1")
    nc.vector.tensor_reduce(s1, etc, AX.X, OP.add)
    rs1 = small.tile([128, G], F32, tag="rs1")
    nc.vector.reciprocal(rs1, s1)
    for bt in range(G):
        nc.vector.tensor_scalar(
            tc_all[:, bt, :], etc[:, bt, :], rs1[:, bt:bt + 1], None, OP.mult)

    # tgt = softmax(true_confs) over each group
    et2 = small.tile([128, G, M], F32, tag="et2")
    nc.scalar.activation(et2.rearrange("p g m -> p (g m)"),
                         tc_all.rearrange("p g m -> p (g m)"), AF.Exp)
    s2 = small.tile([128, G], F32, tag="s2")
    nc.vector.tensor_reduce(s2, et2, AX.X, OP.add)
    rs2 = small.tile([128, G], F32, tag="rs2")
    nc.vector.reciprocal(rs2, s2)
    tgt = small.tile([128, G, M], F32, tag="tgt")
    for bt in range(G):
        nc.vector.tensor_scalar(
            tgt[:, bt, :], et2[:, bt, :], rs2[:, bt:bt + 1], None, OP.mult)

    # bce parts: x*(1-t) and ln(1+exp(-x)), x = wm_s
    omt = small.tile([128, G * M], F32, tag="omt")
    nc.vector.tensor_scalar(omt, tgt.rearrange("p g m -> p (g m)"),
                            -1.0, 1.0, OP.mult, OP.add)
    junk2 = small.tile([128, G * M], F32, tag="junk2")
    nc.vector.scalar_tensor_tensor(
        out=junk2, in0=omt, scalar=1.0, in1=wms_f,
        op0=OP.mult, op1=OP.mult, accum_out=part8[:, 1:2])
    enx = small.tile([128, G * M], F32, tag="enx")
    nc.scalar.activation(enx, wms_f, AF.Exp, scale=-1.0)
    junk3 = small.tile([128, G * M], F32, tag="junk3")
    nc.scalar.activation(junk3, enx, AF.Ln, bias=1.0,
                         accum_out=part8[:, 2:3])

    nc.sync.dma_start(
        out=tc_out[:].rearrange("(g p) m -> p g m", p=128), in_=tc_all)

    # ensemble loss partials: etrue - ln(esum)
    lnes = singles.tile([64, T // 8], F32)
    nc.scalar.activation(lnes, esum, AF.Ln)
    enscol = singles.tile([64, T // 8], F32)
    nc.vector.tensor_tensor(out=enscol, in0=etrue, in1=lnes, op=OP.subtract)
    nc.vector.tensor_reduce(part8[0:64, 3:4], enscol, AX.X, OP.add)
    ones = singles.tile([128, 1], F32)
    nc.vector.memset(ones, 1.0)
    ps_p = spsum.tile([1, 8], F32, tag="tps")
    nc.tensor.matmul(ps_p, ones, part8, start=True, stop=True)
    psb = singles.tile([1, 8], F32)
    nc.vector.tensor_copy(out=psb, in_=ps_p)
    nc.sync.dma_start(out=partials[:], in_=psb)
    _zpsum_cm.__exit__(None, None, None)


def make_core_inputs(x_in, y_pred, labels, model_emb, w1_w, w1_b, w2_w, w2_b,
                     w3_w, w3_b):
    """Host-side shard + layout prep. Returns list of per-core input dicts."""
    w1T = np.ascontiguousarray(w1_w.T)
    w2T = np.ascontiguousarray(w2_w.T)
    membT = np.ascontiguousarray(model_emb.T)
    w3T = np.ascontiguousarray(w3_w.T)
    b1 = np.ascontiguousarray(w1_b.reshape(EMB, 1))
    b2 = np.ascontiguousarray(w2_b.reshape(EMB, 1))
    b3 = np.ascontiguousarray(w3_b.reshape(M, 1))
    iota_c = np.tile(np.arange(C, dtype=np.float32), (128, 1))
    ident = np.eye(128, dtype=np.float32)
    # bdm[p, j, c] = 1 where c == j*8 + p//16 (block-diag column mask)
    bdm = np.zeros((128, 8, 64), dtype=np.float32)
    pidx = np.arange(128)
    for j in range(8):
        bdm[pidx, j, j * 8 + pidx // 16] = 1.0

    in_maps = []
    for cid in range(NCORES):
        sl = slice(cid * R, (cid + 1) * R)
        ls = labels[sl].astype(np.int64)
        # lab_u16[g*16 + j, 2*sb] = j*C + labels[(sb*16+j)*8 + g]: the j-th
        # index of group g for super-tile sb (16 gathers per IndirectCopy;
        # even columns keep the uint16 slice 4-byte aligned)
        lab_u16 = np.zeros((128, T // 8), dtype=np.uint16)
        a = ls.reshape(T // 16, 16, 8)              # [sb, j, g]
        val = a.transpose(2, 1, 0) + (np.arange(16) * C)[None, :, None]
        lab_u16[:, ::2] = val.reshape(128, T // 16).astype(np.uint16)
        lab_g64 = np.ascontiguousarray(
            ls.reshape(T // 8, 64).T.astype(np.float32))        # [64, T//8]
        in_maps.append({
            "xT": np.ascontiguousarray(x_in[sl].T),
            "yy": np.ascontiguousarray(y_pred[sl]),
            "w1T": w1T, "w2T": w2T, "membT": membT, "w3T": w3T,
            "b1": b1, "b2": b2, "b3": b3,
            "iota_c": iota_c, "lab_u16": lab_u16, "lab_g64": lab_g64,
            "ident": ident, "bdm": bdm,
        })
    return in_maps


_NC_CACHE = None


def get_nc():
    global _NC_CACHE
    if _NC_CACHE is None:
        _NC_CACHE = build_nc()
    return _NC_CACHE


def kernel(x_in, y_pred, labels, model_emb, w1_w, w1_b, w2_w, w2_b, w3_w,
           w3_b, class_type=C, **run_kwargs):
    x_in = np.asarray(x_in, dtype=np.float32)
    y_pred = np.asarray(y_pred, dtype=np.float32)
    labels = np.asarray(labels)
    model_emb = np.asarray(model_emb, dtype=np.float32)
    w1_w = np.asarray(w1_w, dtype=np.float32)
    w1_b = np.asarray(w1_b, dtype=np.float32)
    w2_w = np.asarray(w2_w, dtype=np.float32)
    w2_b = np.asarray(w2_b, dtype=np.float32)
    w3_w = np.asarray(w3_w, dtype=np.float32)
    w3_b = np.asarray(w3_b, dtype=np.float32)

    nc = get_nc()
    in_maps = make_core_inputs(x_in, y_pred, labels, model_emb, w1_w, w1_b,
                               w2_w, w2_b, w3_w, w3_b)
    res = run_bass_kernel_spmd(nc, in_maps, core_ids=list(range(NCORES)),
                               **run_kwargs)
    results = res.results

    ems_out_post = np.concatenate([r["post_out"] for r in results], axis=0)
    wm_s = np.concatenate([r["wms_out"] for r in results], axis=0)
    true_confs = np.concatenate([r["tc_out"] for r in results], axis=0)

    p = np.stack([r["partials"][0] for r in results]).astype(np.float64)
    child_loss = np.float32(p[:, 0].sum() / (B * M))
    confidence_loss = np.float32((p[:, 1].sum() + p[:, 2].sum()) / (B * M))
    ensemble_loss = np.float32(-p[:, 3].sum() / B)

    out = (ems_out_post, child_loss, confidence_loss, ensemble_loss, wm_s,
           true_confs)
    if run_kwargs:
        return out, res
    return out


# revision 53
# speedup vs baseline: 1.3893x; 1.3893x over previous
"""Trainium2 Bass kernel for the nn_Ensemble_net MoE-routing problem.

Strategy: data-parallel over batch B=4096 across 8 NeuronCores (512 rows each).
Per core, y_pred is streamed once in 64 tiles laid out [(8 samples x 16
experts) partitions, 1000 classes]:
  - ACT computes exp with fused free-dim accumulation (-> logsumexp).
  - GPSIMD indirect_copy gathers y[b, m, labels[b]] 16 tiles at a time (each
    16-partition group shares one sample per index slot).
  - PE computes both weighted combines (ems_out / ems_out_post); 8 tiles
    accumulate into one [128, C] PSUM tile (64 b-major ens rows + 64 b-major
    post rows) via zero-padded block-diagonal stationary operands, so the
    PSUM->SBUF eviction costs one DVE copy per 8 tiles.
Routing MLP (w1/w2/w3) runs on PE in transposed space; rowwise softmax /
top-k / loss math runs on [128,16] b-major tiles (DVE max8 for top-k).
Losses are reduced to per-core partial sums; the host combines them.

build_nc(niter=N) emits the whole body N times into one NEFF — used by the
differential timing harness (dispatch overhead through the axon tunnel is
~90 ms, far larger than the kernel itself).
"""

import os
import numpy as np
from contextlib import ExitStack

ABLATE = set(os.environ.get("KERNEL_ABLATE", "").split(","))
# 1 = open streaming SBUF pools around the routing phase (y prefetch overlaps
# routing); 0 = phase-scoped pools (original structure, epool bufs=3)
PREOPEN = os.environ.get("KERNEL_PREOPEN", "1") == "1"
# y-DMA slices per 16-tile super-tile (must divide 16)
YSLICES = int(os.environ.get("KERNEL_YSLICES", "4"))

import concourse.bass as bass
import concourse.bacc as bacc
import concourse.tile as tile
from concourse import mybir
from concourse.bass_utils import run_bass_kernel_spmd

F32 = mybir.dt.float32
U16 = mybir.dt.uint16
AX = mybir.AxisListType
OP = mybir.AluOpType
AF = mybir.ActivationFunctionType

B, M, C, NUM_LOCAL, EMB, D = 4096, 16, 1000, 4, 100, 1000
NCORES = 8
R = B // NCORES          # 512 rows per core
T = R // 8               # 64 main-loop tiles per core
G = R // 128             # 4 b-major groups per core
DCH = 8                  # contraction chunks for D=1000 (125 partitions each)
DP = D // DCH            # 125
EPS = 1e-12


def build_nc(niter=1):
    nc = bacc.Bacc("TRN2", target_bir_lowering=False, debug=False)

    # ---- DRAM I/O ----
    xT = nc.dram_tensor("xT", [D, R], F32, kind="ExternalInput")
    yy = nc.dram_tensor("yy", [R, M * C], F32, kind="ExternalInput")
    w1T = nc.dram_tensor("w1T", [D, EMB], F32, kind="ExternalInput")
    w2T = nc.dram_tensor("w2T", [D, EMB], F32, kind="ExternalInput")
    membT = nc.dram_tensor("membT", [D, M], F32, kind="ExternalInput")
    w3T = nc.dram_tensor("w3T", [M, M], F32, kind="ExternalInput")
    b1 = nc.dram_tensor("b1", [EMB, 1], F32, kind="ExternalInput")
    b2 = nc.dram_tensor("b2", [EMB, 1], F32, kind="ExternalInput")
    b3 = nc.dram_tensor("b3", [M, 1], F32, kind="ExternalInput")
    iota_c = nc.dram_tensor("iota_c", [128, C], F32, kind="ExternalInput")
    lab_u16 = nc.dram_tensor("lab_u16", [128, T // 8], U16, kind="ExternalInput")
    lab_g64 = nc.dram_tensor("lab_g64", [64, T // 8], F32, kind="ExternalInput")
    ident = nc.dram_tensor("ident", [128, 128], F32, kind="ExternalInput")
    bdm = nc.dram_tensor("bdm", [128, 8, 64], F32, kind="ExternalInput")

    post_out = nc.dram_tensor("post_out", [R, C], F32, kind="ExternalOutput")
    wms_out = nc.dram_tensor("wms_out", [R, M], F32, kind="ExternalOutput")
    tc_out = nc.dram_tensor("tc_out", [R, M], F32, kind="ExternalOutput")
    partials = nc.dram_tensor("partials", [1, 8], F32, kind="ExternalOutput")

    with tile.TileContext(nc) as tc, ExitStack() as ctx:
        singles = ctx.enter_context(tc.tile_pool(name="singles", bufs=1))
        small = ctx.enter_context(tc.tile_pool(name="small", bufs=4))
        dram = ctx.enter_context(tc.tile_pool(name="dram", bufs=1, space="DRAM"))

        # ---- constants (loaded once) ----
        iota_sb = singles.tile([128, C], F32)
        nc.sync.dma_start(out=iota_sb, in_=iota_c[:])
        lab16_sb = singles.tile([128, T // 8], U16)
        nc.sync.dma_start(out=lab16_sb, in_=lab_u16[:])
        labg64_sb = singles.tile([64, T // 8], F32)
        nc.sync.dma_start(out=labg64_sb, in_=lab_g64[:])
        ident_sb = singles.tile([128, 128], F32)
        nc.sync.dma_start(out=ident_sb, in_=ident[:])
        bdm_sb = singles.tile([128, 8, 64], F32)
        nc.sync.dma_start(out=bdm_sb, in_=bdm[:])
        b1_sb = singles.tile([EMB, 1], F32)
        nc.sync.dma_start(out=b1_sb, in_=b1[:])
        b2_sb = singles.tile([EMB, 1], F32)
        nc.sync.dma_start(out=b2_sb, in_=b2[:])
        b3_sb = singles.tile([M, 1], F32)
        nc.sync.dma_start(out=b3_sb, in_=b3[:])
        w3_sb = singles.tile([M, M], F32)
        nc.sync.dma_start(out=w3_sb, in_=w3T[:])

        for it in range(niter):
            emit_body(nc, tc, it, singles, small, dram,
                      iota_sb, lab16_sb, labg64_sb, ident_sb, bdm_sb,
                      b1_sb, b2_sb, b3_sb, w3_sb,
                      xT, yy, membT, w1T, w2T,
                      post_out, wms_out, tc_out, partials)

    nc.compile()
    return nc


def emit_body(nc, tc, it, singles, small, dram,
              iota_sb, lab16_sb, labg64_sb, ident_sb, bdm_sb,
              b1_sb, b2_sb, b3_sb, w3_sb,
              xT, yy, membT, w1T, w2T,
              post_out, wms_out, tc_out, partials):
    # ---- per-iteration persistent accumulators ----
    sume = singles.tile([128, T], F32)    # sum(exp(y)) per (sample, expert)
    ytr = singles.tile([128, T], F32)     # y[b, m, labels[b]]
    etrue = singles.tile([64, T // 8], F32)   # ems_out[b, labels[b]]
    esum = singles.tile([64, T // 8], F32)    # sum(exp(ems_out[b, :]))
    wms_all = singles.tile([128, G, M], F32)
    tc_all = singles.tile([128, G, M], F32)
    wmcol = singles.tile([128, T], F32)
    pwcol = singles.tile([128, T], F32)

    # =========================================================
    # Phase 1: routing MLP in transposed space
    #   featT = relu(w1 @ x.T + b1)    [EMB, R]
    #   embT  = relu(w2 @ memb.T + b2) [EMB, M]
    #   scoresT[m, b], weightsT = softplus(w3 @ scoresT + b3) [M, R]
    # The streaming SBUF pools open FIRST so y prefetch / exp / gather can
    # run concurrently with routing (only the combine matmuls wait on
    # wmcol/pwcol). PSUM pools stay phase-scoped (8-bank budget).
    # =========================================================
    if PREOPEN:
        _mainpools = [
            tc.tile_pool(name=f"ypool{it}", bufs=2),
            tc.tile_pool(name=f"epool{it}", bufs=2),
            tc.tile_pool(name=f"bdpool{it}", bufs=3),
            tc.tile_pool(name=f"cbpool{it}", bufs=2),
        ]
        ypool, epool, bdpool, cbpool = [p.__enter__() for p in _mainpools]
    with tc.tile_pool(name=f"routing{it}", bufs=1) as routing, \
         tc.tile_pool(name=f"rpsum{it}", bufs=1, space="PSUM") as rpsum, \
         tc.tile_pool(name=f"tpsum{it}", bufs=2, space="PSUM") as spsum:
        x_sb = routing.tile([DP, DCH, R], F32)
        nc.sync.dma_start(
            out=x_sb, in_=xT[:].rearrange("(c p) r -> p c r", p=DP))
        w1_sb = routing.tile([DP, DCH, EMB], F32)
        nc.sync.dma_start(
            out=w1_sb, in_=w1T[:].rearrange("(c p) e -> p c e", p=DP))
        w2_sb = routing.tile([DP, DCH, EMB], F32)
        nc.sync.dma_start(
            out=w2_sb, in_=w2T[:].rearrange("(c p) e -> p c e", p=DP))
        me_sb = routing.tile([DP, DCH, M], F32)
        nc.sync.dma_start(
            out=me_sb, in_=membT[:].rearrange("(c p) m -> p c m", p=DP))

        ps_feat = rpsum.tile([EMB, R], F32)
        for c in range(DCH):
            nc.tensor.matmul(ps_feat, w1_sb[:, c, :], x_sb[:, c, :],
                             start=(c == 0), stop=(c == DCH - 1))
        featT = routing.tile([EMB, R], F32)
        nc.vector.tensor_scalar(featT, ps_feat, b1_sb[:], 0.0, OP.add, OP.max)

        ps_emb = rpsum.tile([EMB, M], F32)
        for c in range(DCH):
            nc.tensor.matmul(ps_emb, w2_sb[:, c, :], me_sb[:, c, :],
                             start=(c == 0), stop=(c == DCH - 1))
        embT = routing.tile([EMB, M], F32)
        nc.vector.tensor_scalar(embT, ps_emb, b2_sb[:], 0.0, OP.add, OP.max)

        ps_sc = rpsum.tile([M, R], F32)
        nc.tensor.matmul(ps_sc, embT, featT, start=True, stop=True)
        scoresT = routing.tile([M, R], F32)
        nc.vector.tensor_copy(out=scoresT, in_=ps_sc)

        ps_u = rpsum.tile([M, R], F32)
        nc.tensor.matmul(ps_u, w3_sb, scoresT, start=True, stop=True)
        u_sb = routing.tile([M, R], F32)
        nc.vector.tensor_scalar(u_sb, ps_u, b3_sb[:], None, OP.add)

        # stable softplus(u) = relu(u) + ln(1 + exp(-|u|))
        nu = routing.tile([M, R], F32)
        nc.vector.tensor_scalar(nu, u_sb, -1.0, None, OP.mult)
        au = routing.tile([M, R], F32)
        nc.vector.tensor_tensor(out=au, in0=u_sb, in1=nu, op=OP.max)
        eau = routing.tile([M, R], F32)
        nc.scalar.activation(eau, au, AF.Exp, scale=-1.0)
        l1p = routing.tile([M, R], F32)
        nc.scalar.activation(l1p, eau, AF.Ln, bias=1.0)
        ru = routing.tile([M, R], F32)
        nc.vector.tensor_scalar(ru, u_sb, 0.0, None, OP.max)
        weightsT = routing.tile([M, R], F32)
        nc.vector.tensor_tensor(out=weightsT, in0=ru, in1=l1p, op=OP.add)

        # =====================================================
        # Phase 2: rowwise weight math on b-major [128, M] tiles
        # =====================================================
        scr_w = dram.tile([R, M], F32)
        scr_p = dram.tile([R, M], F32)
        for bt in range(G):
            ps_t = spsum.tile([128, M], F32, tag="tps")
            nc.tensor.transpose(
                ps_t, weightsT[:, bt * 128:(bt + 1) * 128], ident_sb[:M, :M])
            w_b = small.tile([128, M], F32)
            nc.vector.tensor_copy(out=w_b, in_=ps_t)

            srow = small.tile([128, 1], F32, tag="srow")
            nc.vector.tensor_reduce(srow, w_b, AX.X, OP.add)
            nc.vector.tensor_scalar(srow, srow, EPS, None, OP.max)
            rs = small.tile([128, 1], F32, tag="rs")
            nc.vector.reciprocal(rs, srow)
            wm_b = small.tile([128, M], F32, tag="wm_b")
            nc.vector.tensor_scalar(wm_b, w_b, rs[:], None, OP.mult)
            nc.sync.dma_start(out=scr_w[bt * 128:(bt + 1) * 128, :], in_=wm_b)

            # top-4 sparsification via max8 threshold
            top8 = small.tile([128, 8], F32, tag="top8")
            nc.vector.max(top8, w_b)
            pwr = small.tile([128, M], F32, tag="pwr")
            spost = small.tile([128, 1], F32, tag="spost")
            nc.vector.scalar_tensor_tensor(
                out=pwr, in0=w_b, scalar=top8[:, NUM_LOCAL - 1:NUM_LOCAL],
                in1=w_b, op0=OP.is_ge, op1=OP.mult, accum_out=spost)
            nc.vector.tensor_scalar(spost, spost, EPS, None, OP.max)
            rsp = small.tile([128, 1], F32, tag="rsp")
            nc.vector.reciprocal(rsp, spost)
            pw_b = small.tile([128, M], F32, tag="pw_b")
            nc.vector.tensor_scalar(pw_b, pwr, rsp[:], None, OP.mult)
            nc.sync.dma_start(out=scr_p[bt * 128:(bt + 1) * 128, :], in_=pw_b)

            # wm_s = softmax(weighted_mat) over experts
            ewm = small.tile([128, M], F32, tag="ewm")
            sew = small.tile([128, 1], F32, tag="sew")
            nc.scalar.activation(ewm, wm_b, AF.Exp, accum_out=sew)
            rsew = small.tile([128, 1], F32, tag="rsew")
            nc.vector.reciprocal(rsew, sew)
            nc.vector.tensor_scalar(
                wms_all[:, bt, :], ewm, rsew[:], None, OP.mult)

        nc.sync.dma_start(
            out=wms_out[:].rearrange("(g p) m -> p g m", p=128),
            in_=wms_all)

        # relayout weighted_mat / post_weights into per-tile PE columns:
        # wmcol[(b8*16+m), t] = wm[t*8+b8, m]
        for src, dst in ((scr_w, wmcol), (scr_p, pwcol)):
            v = routing.tile([T, 128], F32, tag="wcolv")
            nc.sync.dma_start(
                out=v, in_=src[:].rearrange("(t x) m -> t (x m)", t=T))
            ps_c = spsum.tile([128, T], F32, tag="tps")
            nc.tensor.transpose(ps_c, v, ident_sb[:T, :T])
            nc.vector.tensor_copy(out=dst, in_=ps_c)

    # =========================================================
    # Phase 3: main streaming loop over y, 4 super-tiles of 16 tiles;
    # each half-super-tile (8 tiles) accumulates its combines into one
    # [128, C] PSUM tile: rows 0:64 ens (b-major), 64:128 post (b-major).
    # =========================================================
    if not PREOPEN:
        _mainpools = [
            tc.tile_pool(name=f"ypool{it}", bufs=2),
            tc.tile_pool(name=f"epool{it}", bufs=3),
            tc.tile_pool(name=f"bdpool{it}", bufs=3),
            tc.tile_pool(name=f"cbpool{it}", bufs=2),
        ]
        ypool, epool, bdpool, cbpool = [p.__enter__() for p in _mainpools]
    with tc.tile_pool(name=f"cpsum{it}", bufs=4, space="PSUM") as cpsum:
        for sb in range(T // 16):
            ysup = ypool.tile([128, 16, C], F32)
            # DMA slices for pipelining
            jq = 16 // YSLICES
            for q in range(YSLICES):
                nc.sync.dma_start(
                    out=ysup[:, q * jq:(q + 1) * jq, :],
                    in_=yy[sb * 128 + q * jq * 8:
                           sb * 128 + (q + 1) * jq * 8, :]
                    .rearrange("(j b) (m c) -> (b m) j c", j=jq, c=C))

            # label gather, 16 tiles at once: one index per partition of
            # each 16-partition group; index j gathers tile j's value.
            if "gather" not in ABLATE:
                nc.gpsimd.indirect_copy(
                    out=ytr[:, sb * 16:(sb + 1) * 16],
                    data=ysup.rearrange("p j c -> p (j c)"),
                    idxs=lab16_sb[:, 2 * sb:2 * sb + 1],
                    i_know_ap_gather_is_preferred=True)
            else:
                nc.vector.memset(ytr[:, sb * 16:(sb + 1) * 16], 0.5)

            for h2 in range(2):   # psum batches of 8 tiles
                h = sb * 2 + h2
                cps = cpsum.tile([128, C], F32)
                for jj in range(8):
                    j = h2 * 8 + jj
                    t = sb * 16 + j
                    y_t = ysup[:, j, :]

                    if "comb" not in ABLATE:
                        # per-tile stationary operand, zero outside its
                        # columns: ens jj*8:(jj+1)*8, post 64+jj*8:...
                        bd = bdpool.tile([128, 128], F32)
                        nc.vector.tensor_scalar(
                            bd[:, 0:64], bdm_sb[:, jj, :],
                            wmcol[:, t:t + 1], None, OP.mult)
                        nc.vector.tensor_scalar(
                            bd[:, 64:128], bdm_sb[:, jj, :],
                            pwcol[:, t:t + 1], None, OP.mult)

                        nc.tensor.matmul(cps[:, 0:512], bd, y_t[:, 0:512],
                                         start=(jj == 0), stop=(jj == 7),
                                         skip_group_check=True)
                        nc.tensor.matmul(cps[:, 512:C], bd, y_t[:, 512:C],
                                         start=(jj == 0), stop=(jj == 7),
                                         skip_group_check=True)
                    elif jj == 0:
                        nc.vector.memset(cps, 0.25)

                    # exp + fused row-sum (for logsumexp)
                    if "exp" not in ABLATE:
                        esc = epool.tile([128, C], F32, tag="esc")
                        nc.scalar.activation(esc, y_t, AF.Exp,
                                             accum_out=sume[:, t:t + 1])
                    elif jj == 0:
                        nc.vector.memset(sume[:, t:t + 8], 1.0)

                # evict the batch: one copy for 64 ens + 64 post rows
                comb = cbpool.tile([128, C], F32)
                if "evict" in ABLATE:
                    nc.vector.memset(comb[:, 0:8], 0.25)
                else:
                    nc.vector.tensor_copy(out=comb, in_=cps)
                    nc.sync.dma_start(
                        out=post_out[h * 64:(h + 1) * 64, :],
                        in_=comb[64:128, :])

                # ensemble-loss pieces on the 64 b-major ens rows
                esc2 = epool.tile([128, C], F32, tag="esc")
                nc.vector.scalar_tensor_tensor(
                    out=esc2[0:64, :], in0=iota_sb[0:64, :],
                    scalar=labg64_sb[:, h:h + 1],
                    in1=comb[0:64, :], op0=OP.is_equal, op1=OP.mult,
                    accum_out=etrue[:, h:h + 1])
                esc3 = epool.tile([128, C], F32, tag="esc")
                nc.scalar.activation(esc3[0:64, :], comb[0:64, :], AF.Exp,
                                     accum_out=esum[:, h:h + 1])

    for p_ in reversed(_mainpools):
        p_.__exit__(None, None, None)

    # =========================================================
    # Phase 4: tail — logsumexp, relayout, losses
    # =========================================================
    if "tail" in ABLATE:   # timing-ablation stub, never used in production
        junkt = small.tile([128, 8], F32, tag="junkt")
        nc.vector.tensor_copy(out=junkt[:, 0:1], in_=sume[:, 0:1])
        nc.vector.tensor_copy(out=junkt[:, 1:2], in_=ytr[:, 0:1])
        nc.sync.dma_start(out=partials[:], in_=junkt[0:1, :])
        return
    _zpsum_cm = tc.tile_pool(name=f"zpsum{it}", bufs=1, space="PSUM")
    spsum = _zpsum_cm.__enter__()
    pk = singles.tile([128, 128], F32)
    nc.scalar.activation(pk[:, 0:T], sume, AF.Ln)
    nc.vector.tensor_copy(out=pk[:, T:2 * T], in_=ytr)
    ps_pk = spsum.tile([128, 128], F32, tag="tps")
    nc.tensor.transpose(ps_pk, pk, ident_sb)
    pkt = singles.tile([128, 128], F32)
    nc.vector.tensor_copy(out=pkt, in_=ps_pk)
    scr_pk = dram.tile([128, 128], F32)
    nc.sync.dma_start(out=scr_pk, in_=pkt)

    lse_bm = singles.tile([128, G, M], F32)
    nc.sync.dma_start(
        out=lse_bm,
        in_=scr_pk[0:T, :].rearrange(
            "(g th) (bl m) -> (th bl) g m", g=G, bl=8))
    ytr_bm = singles.tile([128, G, M], F32)
    nc.sync.dma_start(
        out=ytr_bm,
        in_=scr_pk[T:2 * T, :].rearrange(
            "(g th) (bl m) -> (th bl) g m", g=G, bl=8))

    # batched [128, G*M] tail math; loss partials accumulate straight into
    # per-partition columns of part8 via fused accum_out over all 64 cols
    part8 = singles.tile([128, 8], F32)
    nc.vector.memset(part8, 0.0)
    lse_f = lse_bm.rearrange("p g m -> p (g m)")
    ytr_f = ytr_bm.rearrange("p g m -> p (g m)")
    wms_f = wms_all.rearrange("p g m -> p (g m)")

    epl = small.tile([128, G * M], F32, tag="epl")
    nc.vector.tensor_tensor(out=epl, in0=lse_f, in1=ytr_f, op=OP.subtract)
    junk = small.tile([128, G * M], F32, tag="junk")
    nc.vector.scalar_tensor_tensor(
        out=junk, in0=epl, scalar=1.0, in1=wms_f,
        op0=OP.mult, op1=OP.mult, accum_out=part8[:, 0:1])

    dd = small.tile([128, G * M], F32, tag="dd")
    nc.vector.tensor_tensor(out=dd, in0=ytr_f, in1=lse_f, op=OP.subtract)
    tcp = small.tile([128, G * M], F32, tag="tcp")
    nc.scalar.activation(tcp, dd, AF.Exp)

    # true_confs = softmax(tcp) over each 16-col group
    etc = small.tile([128, G, M], F32, tag="etc")
    nc.scalar.activation(etc.rearrange("p g m -> p (g m)"), tcp, AF.Exp)
    s1 = small.tile([128, G], F32, tag="s1")
    nc.vector.tensor_reduce(s1, etc, AX.X, OP.add)
    rs1 = small.tile([128, G], F32, tag="rs1")
    nc.vector.reciprocal(rs1, s1)
    for bt in range(G):
        nc.vector.tensor_scalar(
            tc_all[:, bt, :], etc[:, bt, :], rs1[:, bt:bt + 1], None, OP.mult)

    # tgt = softmax(true_confs) over each group
    et2 = small.tile([128, G, M], F32, tag="et2")
    nc.scalar.activation(et2.rearrange("p g m -> p (g m)"),
                         tc_all.rearrange("p g m -> p (g m)"), AF.Exp)
    s2 = small.tile([128, G], F32, tag="s2")
    nc.vector.tensor_reduce(s2, et2, AX.X, OP.add)
    rs2 = small.tile([128, G], F32, tag="rs2")
    nc.vector.reciprocal(rs2, s2)
    tgt = small.tile([128, G, M], F32, tag="tgt")
    for bt in range(G):
        nc.vector.tensor_scalar(
            tgt[:, bt, :], et2[:, bt, :], rs2[:, bt:bt + 1], None, OP.mult)

    # bce parts: x*(1-t) and ln(1+exp(-x)), x = wm_s
    omt = small.tile([128, G * M], F32, tag="omt")
    nc.vector.tensor_scalar(omt, tgt.rearrange("p g m -> p (g m)"),
                            -1.0, 1.0, OP.mult, OP.add)
    junk2 = small.tile([128, G * M], F32, tag="junk2")
    nc.vector.scalar_tensor_tensor(
        out=junk2, in0=omt, scalar=1.0, in1=wms_f,
        op0=OP.mult, op1=OP.mult, accum_out=part8[:, 1:2])
    enx = small.tile([128, G * M], F32, tag="enx")
    nc.scalar.activation(enx, wms_f, AF.Exp, scale=-1.0)
    junk3 = small.tile([128, G * M], F32, tag="junk3")
    nc.scalar.activation(junk3, enx, AF.Ln, bias=1.0,
                         accum_out=part8[:, 2:3])

    nc.sync.dma_start(
        out=tc_out[:].rearrange("(g p) m -> p g m", p=128), in_=tc_all)

    # ensemble loss partials: etrue - ln(esum)
    lnes = singles.tile([64, T // 8], F32)
    nc.scalar.activation(lnes, esum, AF.Ln)
    enscol = singles.tile([64, T // 8], F32)
    nc.vector.tensor_tensor(out=enscol, in0=etrue, in1=lnes, op=OP.subtract)
    nc.vector.tensor_reduce(part8[0:64, 3:4], enscol, AX.X, OP.add)
    ones = singles.tile([128, 1], F32)
    nc.vector.memset(ones, 1.0)
    ps_p = spsum.tile([1, 8], F32, tag="tps")
    nc.tensor.matmul(ps_p, ones, part8, start=True, stop=True)
    psb = singles.tile([1, 8], F32)
    nc.vector.tensor_copy(out=psb, in_=ps_p)
    nc.sync.dma_start(out=partials[:], in_=psb)
    _zpsum_cm.__exit__(None, None, None)


def make_core_inputs(x_in, y_pred, labels, model_emb, w1_w, w1_b, w2_w, w2_b,
                     w3_w, w3_b):
    """Host-side shard + layout prep. Returns list of per-core input dicts."""
    w1T = np.ascontiguousarray(w1_w.T)
    w2T = np.ascontiguousarray(w2_w.T)
    membT = np.ascontiguousarray(model_emb.T)
    w3T = np.ascontiguousarray(w3_w.T)
    b1 = np.ascontiguousarray(w1_b.reshape(EMB, 1))
    b2 = np.ascontiguousarray(w2_b.reshape(EMB, 1))
    b3 = np.ascontiguousarray(w3_b.reshape(M, 1))
    iota_c = np.tile(np.arange(C, dtype=np.float32), (128, 1))
    ident = np.eye(128, dtype=np.float32)
    # bdm[p, j, c] = 1 where c == j*8 + p//16 (block-diag column mask)
    bdm = np.zeros((128, 8, 64), dtype=np.float32)
    pidx = np.arange(128)
    for j in range(8):
        bdm[pidx, j, j * 8 + pidx // 16] = 1.0

    in_maps = []
    for cid in range(NCORES):
        sl = slice(cid * R, (cid + 1) * R)
        ls = labels[sl].astype(np.int64)
        # lab_u16[g*16 + j, 2*sb] = j*C + labels[(sb*16+j)*8 + g]: the j-th
        # index of group g for super-tile sb (16 gathers per IndirectCopy;
        # even columns keep the uint16 slice 4-byte aligned)
        lab_u16 = np.zeros((128, T // 8), dtype=np.uint16)
        a = ls.reshape(T // 16, 16, 8)              # [sb, j, g]
        val = a.transpose(2, 1, 0) + (np.arange(16) * C)[None, :, None]
        lab_u16[:, ::2] = val.reshape(128, T // 16).astype(np.uint16)
        lab_g64 = np.ascontiguousarray(
            ls.reshape(T // 8, 64).T.astype(np.float32))        # [64, T//8]
        in_maps.append({
            "xT": np.ascontiguousarray(x_in[sl].T),
            "yy": np.ascontiguousarray(y_pred[sl]),
            "w1T": w1T, "w2T": w2T, "membT": membT, "w3T": w3T,
            "b1": b1, "b2": b2, "b3": b3,
            "iota_c": iota_c, "lab_u16": lab_u16, "lab_g64": lab_g64,
            "ident": ident, "bdm": bdm,
        })
    return in_maps


_NC_CACHE = None


def get_nc():
    global _NC_CACHE
    if _NC_CACHE is None:
        _NC_CACHE = build_nc()
    return _NC_CACHE


def kernel(x_in, y_pred, labels, model_emb, w1_w, w1_b, w2_w, w2_b, w3_w,
           w3_b, class_type=C, **run_kwargs):
    x_in = np.asarray(x_in, dtype=np.float32)
    y_pred = np.asarray(y_pred, dtype=np.float32)
    labels = np.asarray(labels)
    model_emb = np.asarray(model_emb, dtype=np.float32)
    w1_w = np.asarray(w1_w, dtype=np.float32)
    w1_b = np.asarray(w1_b, dtype=np.float32)
    w2_w = np.asarray(w2_w, dtype=np.float32)
    w2_b = np.asarray(w2_b, dtype=np.float32)
    w3_w = np.asarray(w3_w, dtype=np.float32)
    w3_b = np.asarray(w3_b, dtype=np.float32)

    nc = get_nc()
    in_maps = make_core_inputs(x_in, y_pred, labels, model_emb, w1_w, w1_b,
                               w2_w, w2_b, w3_w, w3_b)
    res = run_bass_kernel_spmd(nc, in_maps, core_ids=list(range(NCORES)),
                               **run_kwargs)
    results = res.results

    ems_out_post = np.concatenate([r["post_out"] for r in results], axis=0)
    wm_s = np.concatenate([r["wms_out"] for r in results], axis=0)
    true_confs = np.concatenate([r["tc_out"] for r in results], axis=0)

    p = np.stack([r["partials"][0] for r in results]).astype(np.float64)
    child_loss = np.float32(p[:, 0].sum() / (B * M))
    confidence_loss = np.float32((p[:, 1].sum() + p[:, 2].sum()) / (B * M))
    ensemble_loss = np.float32(-p[:, 3].sum() / B)

    out = (ems_out_post, child_loss, confidence_loss, ensemble_loss, wm_s,
           true_confs)
    if run_kwargs:
        return out, res
    return out
